# revision 17
# baseline (speedup 1.0000x reference)
"""Multi-head attention (B=2, S=2048, D=1024, H=16) on 8 Trainium2 cores.

Sharding: core c = b*4 + g handles batch b = c//4 and head-group g = c%4
(4 heads). Tensor-parallel over QKV columns / WO rows; final out-projection
partials summed on host; bv folds into a host-side (bv @ wo) constant since
softmax rows sum to 1.

Per-core kernel (all matmuls fp32r = 11-bit mantissa, fp32 accumulate):
  stage2: qT,kT = (wq|wk)^T x^T [256, S] (+bias per-partition), v [S, 256]
          stored augmented with a ones column per head ([S, 4*65]) so the
          attn@v matmul also produces the softmax row-sums Z.
  stage3: per (q-tile 512, head): S^T[k,q] = kT.T@qT blocks -> exp (scale
          1/8 folded into ACT) -> attnV accumulation [65, 512] whose row 64
          is Z; recip -> K=1 matmul broadcast -> DVE-normalize attn tiles
          in-place -> DMA out [h, k, q]; attnout^T normalized via the same
          broadcast during PSUM->SBUF copy.
  stage4: out^T partial = attnout^T.T @ wo with heads as K=64 chunks.
Host: assemble attn (transpose view copy), sum out partials + bv@wo + bo.
"""

import sys

if "/opt/trn_rl_repo" not in sys.path:
    sys.path.insert(0, "/opt/trn_rl_repo")

import numpy as np

import concourse.bass as bass
import concourse.bacc as bacc
import concourse.tile as tile
import concourse.mybir as mybir
from concourse.bass_utils import run_bass_kernel_spmd

B, S, D = 2, 2048, 1024
H, HD = 16, 64
HPC = 4               # heads per core
GD = HPC * HD         # 256 group dim
P = 128
F32 = mybir.dt.float32
F32R = mybir.dt.float32r
F16 = mybir.dt.float16
AF = mybir.ActivationFunctionType


def _build(loop_iters: int | None = None):
    nc = bacc.Bacc("TRN2", target_bir_lowering=False, debug=False, num_devices=8)

    xt = nc.dram_tensor("xt", [D, S], F32R, kind="ExternalInput").ap()
    wq = nc.dram_tensor("wq", [D, GD], F32R, kind="ExternalInput").ap()
    wk = nc.dram_tensor("wk", [D, GD], F32R, kind="ExternalInput").ap()
    wv = nc.dram_tensor("wv", [D, GD], F32R, kind="ExternalInput").ap()
    wo = nc.dram_tensor("wo", [GD, D], F32R, kind="ExternalInput").ap()
    bq = nc.dram_tensor("bq", [P, 2], F32, kind="ExternalInput").ap()
    bk = nc.dram_tensor("bk", [P, 2], F32, kind="ExternalInput").ap()
    attnT = nc.dram_tensor("attnT", [HPC, S, S], F16, kind="ExternalOutput").ap()
    outp = nc.dram_tensor("outp", [S, D], F32, kind="ExternalOutput").ap()

    with tile.TileContext(nc) as tc:
        def body():
            with tc.tile_pool(name="persist", bufs=1) as pp:
                qT_s = pp.tile([P, 2, S], F32R)     # rows: 2 heads x 64
                # kT zero-padded per head: head h data in its own 64 rows,
                # complementary 64 rows zero -> scores matmul runs at K=128
                # (K=64 matmuls measure ~150ns/instr slower on HW)
                kT_z = pp.tile([P, HPC, S], F32R)
                vaug_s = pp.tile([P, 16, HPC * 65], F16)  # [k-part, ktile, (h,65)]
                # attnout^T stored head-pair stacked: pair j rows 0-63 =
                # head 2j, rows 64-127 = head 2j+1 (odd head arrives via a
                # small SBUF->SBUF DMA, which can cross partitions) -> the
                # out-projection contracts K=128 instead of K=64
                att_s = pp.tile([P, 2, S], F32R)
                wo_s = pp.tile([P, 2, D], F32R)
                bq_s = pp.tile([P, 2], F32)
                bk_s = pp.tile([P, 2], F32)
                ones_f = pp.tile([P, P], F32)
                ones_s = pp.tile([P, P], F32R)

                nc.sync.dma_start(wo_s[:], wo.rearrange("(j p) n -> p j n", p=P))
                nc.sync.dma_start(bq_s[:], bq)
                nc.sync.dma_start(bk_s[:], bk)
                nc.vector.memset(ones_f[:], 1.0)
                nc.vector.tensor_copy(ones_s[:], ones_f[:])
                # ones column per head in v_aug
                nc.vector.tensor_copy(
                    vaug_s.rearrange("p t (h c) -> p t h c", c=65)[:, :, :, 64:65],
                    ones_f[:, 0:64].rearrange("p (t h) -> p t h", h=4).unsqueeze(3),
                )

                # ---------------- stage 2: projections ----------------
                with (
                    tc.tile_pool(name="s2", bufs=1) as s2p,
                    tc.tile_pool(name="ps2", bufs=8, space="PSUM") as ps2p,
                ):
                    xt_s = s2p.tile([P, 8, S], F32R)
                    wq_s = s2p.tile([P, 8, GD], F32R)
                    wk_s = s2p.tile([P, 8, GD], F32R)
                    wv_s = s2p.tile([P, 8, GD], F32R)
                    # per-chunk loads so stage-2 matmuls start after ~1.3MB
                    xt_r = xt.rearrange("(c p) t -> c p t", p=P)
                    wq_r = wq.rearrange("(c p) n -> c p n", p=P)
                    wk_r = wk.rearrange("(c p) n -> c p n", p=P)
                    wv_r = wv.rearrange("(c p) n -> c p n", p=P)
                    for c in range(8):
                        nc.sync.dma_start(xt_s[:, c], xt_r[c])
                        nc.sync.dma_start(wq_s[:, c], wq_r[c])
                        nc.sync.dma_start(wk_s[:, c], wk_r[c])
                        nc.sync.dma_start(wv_s[:, c], wv_r[c])

                    # c-outer: accumulate all 8 (J,t) q/k tiles as chunks arrive
                    qk_ps = {}
                    for J in range(2):
                        for t in range(4):
                            for i in range(2):
                                qk_ps[(J, t, i)] = ps2p.tile([P, 512], F32, tag="qk", name=f"qkps_{J}_{t}_{i}")
                    for c in range(8):
                        for J in range(2):
                            for t in range(4):
                                for i, w_s in enumerate((wq_s, wk_s)):
                                    nc.tensor.matmul(
                                        qk_ps[(J, t, i)][:],
                                        w_s[:, c, J * P:(J + 1) * P],
                                        xt_s[:, c, t * 512:(t + 1) * 512],
                                        start=(c == 0), stop=(c == 7),
                                    )
                    for J in range(2):
                        for t in range(4):
                            tsl = slice(t * 512, (t + 1) * 512)
                            nc.scalar.activation(
                                qT_s[:, J, tsl], qk_ps[(J, t, 0)][:],
                                AF.Identity, bias=bq_s[:, J:J + 1],
                            )
                            # kT per-head padded blocks: data rows + zero rows
                            psk = qk_ps[(J, t, 1)]
                            nc.scalar.activation(
                                kT_z[0:64, 2 * J, tsl], psk[0:64, :],
                                AF.Identity, bias=bk_s[0:64, J:J + 1],
                            )
                            nc.scalar.activation(
                                kT_z[64:P, 2 * J + 1, tsl], psk[64:P, :],
                                AF.Identity, bias=bk_s[64:P, J:J + 1],
                            )
                            nc.vector.tensor_scalar_mul(
                                kT_z[64:P, 2 * J, tsl], psk[64:P, :], 0.0)
                            nc.vector.tensor_scalar_mul(
                                kT_z[0:64, 2 * J + 1, tsl], psk[0:64, :], 0.0)
                    for t in range(16):
                        ps = ps2p.tile([P, 512], F32, tag="qk")
                        psv = ps[:, 0:GD]
                        for c in range(8):
                            nc.tensor.matmul(
                                psv,
                                xt_s[:, c, t * P:(t + 1) * P],
                                wv_s[:, c, :],
                                start=(c == 0), stop=(c == 7),
                            )
                        nc.vector.tensor_copy(
                            vaug_s.rearrange("p t (h c) -> p t h c", c=65)[:, t, :, 0:64],
                            psv.rearrange("p (h c) -> p h c", c=64),
                        )

                # ---------------- stage 3 + 4 ----------------
                with (
                    tc.tile_pool(name="slab", bufs=20) as slabp,
                    tc.tile_pool(name="misc", bufs=3) as miscp,
                    tc.tile_pool(name="pss", bufs=2, space="PSUM") as pssp,
                    tc.tile_pool(name="pso", bufs=2, space="PSUM") as psop,
                    tc.tile_pool(name="psb", bufs=1, space="PSUM") as psbp,
                    tc.tile_pool(name="ps4", bufs=1, space="PSUM") as ps4p,
                ):
                    for qt in range(4):
                        q0 = qt * 512
                        for h in range(4):
                            hj = h // 2
                            ps_o = psop.tile([65, 512], F32)
                            g_tiles = []
                            for g in range(8):
                                ps_s = pssp.tile([P, 1024], F32)
                                for j2 in range(2):
                                    kt = g * 2 + j2
                                    nc.tensor.matmul(
                                        ps_s[:, j2 * 512:(j2 + 1) * 512],
                                        kT_z[:, h, kt * P:(kt + 1) * P],
                                        qT_s[:, hj, q0:q0 + 512],
                                        start=True, stop=True,
                                    )
                                eg = slabp.tile([P, 1024], F16, tag="slab")
                                nc.scalar.activation(eg[:], ps_s[:], AF.Exp, scale=0.125)
                                g_tiles.append(eg)
                                for j2 in range(2):
                                    kt = g * 2 + j2
                                    nc.tensor.matmul(
                                        ps_o[:],
                                        vaug_s[:, kt, h * 65:(h + 1) * 65],
                                        eg[:, j2 * 512:(j2 + 1) * 512],
                                        start=(kt == 0), stop=(kt == 15),
                                    )
                            # Z -> recip -> K=1 matmul broadcast to all partitions
                            rc = miscp.tile([P, 512], F32R, tag="rc")
                            with nc.allow_low_precision(reason="recip feeds fp32r bcast matmul"):
                                nc.vector.reciprocal(rc[64:65, :], ps_o[64:65, :])
                            bc_ps = psbp.tile([P, 512], F32)
                            nc.tensor.matmul(bc_ps[:], ones_s[64:65, :], rc[64:65, :],
                                             start=True, stop=True)
                            bc = miscp.tile([P, 512], F16, tag="bc")
                            nc.vector.tensor_copy(bc[:], bc_ps[:])
                            # normalized attn-out^T column block
                            if h % 2 == 0:
                                nc.vector.tensor_tensor(
                                    att_s[0:64, h // 2, q0:q0 + 512],
                                    ps_o[0:64, :], bc[0:64, :],
                                    op=mybir.AluOpType.mult,
                                )
                            else:
                                odd_tmp = miscp.tile([64, 512], F32R, tag="odd")
                                nc.vector.tensor_tensor(
                                    odd_tmp[:], ps_o[0:64, :], bc[0:64, :],
                                    op=mybir.AluOpType.mult,
                                )
                                nc.sync.dma_start(
                                    att_s[64:P, h // 2, q0:q0 + 512], odd_tmp[:])
                            # normalize attn in place and write out
                            # (split across DVE and GPSIMD to balance engines)
                            for g in range(8):
                                eg = g_tiles[g]
                                eng = nc.gpsimd if g % 3 == 2 else nc.vector
                                for j2 in range(2):
                                    kt = g * 2 + j2
                                    eng.tensor_tensor(
                                        eg[:, j2 * 512:(j2 + 1) * 512],
                                        eg[:, j2 * 512:(j2 + 1) * 512],
                                        bc[:],
                                        op=mybir.AluOpType.mult,
                                    )
                                    nc.sync.dma_start(
                                        attnT[h, kt * P:(kt + 1) * P, q0:q0 + 512],
                                        eg[:, j2 * 512:(j2 + 1) * 512],
                                    )
                        # ---- stage 4 for this q tile ----
                        for qs in range(4):
                            for n2 in range(2):
                                ps4 = ps4p.tile([P, 512], F32)
                                for hj in range(2):
                                    nc.tensor.matmul(
                                        ps4[:],
                                        att_s[:, hj, q0 + qs * P:q0 + (qs + 1) * P],
                                        wo_s[:, hj, n2 * 512:(n2 + 1) * 512],
                                        start=(hj == 0), stop=(hj == 1),
                                    )
                                st = miscp.tile([P, 512], F32, tag="st")
                                nc.vector.tensor_copy(st[:], ps4[:])
                                nc.sync.dma_start(
                                    outp[q0 + qs * P:q0 + (qs + 1) * P,
                                         n2 * 512:(n2 + 1) * 512],
                                    st[:],
                                )

        if loop_iters is not None and loop_iters > 1:
            with tc.For_i(0, loop_iters, 1):
                body()
        else:
            body()

    nc.compile()
    return nc


_cache: dict = {}


def _get_nc(loop_iters=None):
    key = loop_iters
    if key not in _cache:
        _cache[key] = _build(loop_iters)
    return _cache[key]


def _make_in_maps(x, wq, bq, wk, bk, wv, wo):
    in_maps = []
    for c in range(8):
        b, g = c // 4, c % 4
        sl = slice(g * GD, (g + 1) * GD)
        in_maps.append({
            "xt": np.ascontiguousarray(x[b].T),
            "wq": np.ascontiguousarray(wq[:, sl]),
            "wk": np.ascontiguousarray(wk[:, sl]),
            "wv": np.ascontiguousarray(wv[:, sl]),
            "wo": np.ascontiguousarray(wo[sl, :]),
            "bq": np.ascontiguousarray(bq[sl].reshape(2, P).T),
            "bk": np.ascontiguousarray(bk[sl].reshape(2, P).T),
        })
    return in_maps


def kernel(x, wq, bq, wk, bk, wv, bv, wo, bo):
    x = np.asarray(x, dtype=np.float32)
    nc = _get_nc()
    in_maps = _make_in_maps(x, wq, bq, wk, bk, wv, wo)
    res = run_bass_kernel_spmd(nc, in_maps, list(range(8))).results

    out = np.zeros((B, S, D), dtype=np.float32)
    attn = np.empty((B, H, S, S), dtype=np.float32)
    for c in range(8):
        b, g = c // 4, c % 4
        out[b] += res[c]["outp"]
        attn[b, g * HPC:(g + 1) * HPC] = res[c]["attnT"].transpose(0, 2, 1)
    out += np.asarray(bv, np.float32) @ np.asarray(wo, np.float32) + np.asarray(bo, np.float32)
    return out, attn


# revision 18
# speedup vs baseline: 1.0601x; 1.0601x over previous
"""Multi-head attention (B=2, S=2048, D=1024, H=16) on 8 Trainium2 cores.

Sharding: core c = b*4 + g handles batch b = c//4 and head-group g = c%4
(4 heads). Tensor-parallel over QKV columns / WO rows; final out-projection
partials summed on host; bv folds into a host-side (bv @ wo) constant since
softmax rows sum to 1.

Per-core kernel (all matmuls fp32r = 11-bit mantissa, fp32 accumulate):
  stage2: qT,kT = (wq|wk)^T x^T [256, S] (+bias per-partition), v [S, 256]
          stored augmented with a ones column per head ([S, 4*65]) so the
          attn@v matmul also produces the softmax row-sums Z.
  stage3: per (q-tile 512, head): S^T[k,q] = kT.T@qT blocks -> exp (scale
          1/8 folded into ACT) -> attnV accumulation [65, 512] whose row 64
          is Z; recip -> K=1 matmul broadcast -> DVE-normalize attn tiles
          in-place -> DMA out [h, k, q]; attnout^T normalized via the same
          broadcast during PSUM->SBUF copy.
  stage4: out^T partial = attnout^T.T @ wo with heads as K=64 chunks.
Host: assemble attn (transpose view copy), sum out partials + bv@wo + bo.
"""

import sys

if "/opt/trn_rl_repo" not in sys.path:
    sys.path.insert(0, "/opt/trn_rl_repo")

import numpy as np

import concourse.bass as bass
import concourse.bacc as bacc
import concourse.tile as tile
import concourse.mybir as mybir
from concourse.bass_utils import run_bass_kernel_spmd

B, S, D = 2, 2048, 1024
H, HD = 16, 64
HPC = 4               # heads per core
GD = HPC * HD         # 256 group dim
P = 128
F32 = mybir.dt.float32
F32R = mybir.dt.float32r
F16 = mybir.dt.float16
AF = mybir.ActivationFunctionType


def _build(loop_iters: int | None = None):
    nc = bacc.Bacc("TRN2", target_bir_lowering=False, debug=False, num_devices=8)

    xt = nc.dram_tensor("xt", [D, S], F32R, kind="ExternalInput").ap()
    wq = nc.dram_tensor("wq", [D, GD], F32R, kind="ExternalInput").ap()
    wk = nc.dram_tensor("wk", [D, GD], F32R, kind="ExternalInput").ap()
    wv = nc.dram_tensor("wv", [D, GD], F32R, kind="ExternalInput").ap()
    wo = nc.dram_tensor("wo", [GD, D], F32R, kind="ExternalInput").ap()
    bq = nc.dram_tensor("bq", [P, 2], F32, kind="ExternalInput").ap()
    bk = nc.dram_tensor("bk", [P, 2], F32, kind="ExternalInput").ap()
    attnT = nc.dram_tensor("attnT", [HPC, S, S], F16, kind="ExternalOutput").ap()
    outp = nc.dram_tensor("outp", [S, D], F32, kind="ExternalOutput").ap()

    with tile.TileContext(nc) as tc:
        def body():
            with tc.tile_pool(name="persist", bufs=1) as pp:
                qT_s = pp.tile([P, 2, S], F32R)     # rows: 2 heads x 64
                # kT zero-padded per head: head h data in its own 64 rows,
                # complementary 64 rows zero -> scores matmul runs at K=128
                # (K=64 matmuls measure ~150ns/instr slower on HW)
                kT_z = pp.tile([P, HPC, S], F32R)
                vaug_s = pp.tile([P, 16, HPC * 65], F16)  # [k-part, ktile, (h,65)]
                # attnout^T stored head-pair stacked: pair j rows 0-63 =
                # head 2j, rows 64-127 = head 2j+1 (odd head arrives via a
                # small SBUF->SBUF DMA, which can cross partitions) -> the
                # out-projection contracts K=128 instead of K=64
                att_s = pp.tile([P, 2, S], F32R)
                wo_s = pp.tile([P, 2, D], F32R)
                bq_s = pp.tile([P, 2], F32)
                bk_s = pp.tile([P, 2], F32)
                ones_f = pp.tile([P, P], F32)
                ones_s = pp.tile([P, P], F32R)

                nc.sync.dma_start(wo_s[:], wo.rearrange("(j p) n -> p j n", p=P))
                nc.sync.dma_start(bq_s[:], bq)
                nc.sync.dma_start(bk_s[:], bk)
                nc.vector.memset(ones_f[:], 1.0)
                nc.vector.tensor_copy(ones_s[:], ones_f[:])
                # ones column per head in v_aug
                nc.vector.tensor_copy(
                    vaug_s.rearrange("p t (h c) -> p t h c", c=65)[:, :, :, 64:65],
                    ones_f[:, 0:64].rearrange("p (t h) -> p t h", h=4).unsqueeze(3),
                )

                # ---------------- stage 2: projections ----------------
                with (
                    tc.tile_pool(name="s2", bufs=1) as s2p,
                    tc.tile_pool(name="ps2", bufs=8, space="PSUM") as ps2p,
                ):
                    xt_s = s2p.tile([P, 8, S], F32R)
                    wq_s = s2p.tile([P, 8, GD], F32R)
                    wk_s = s2p.tile([P, 8, GD], F32R)
                    wv_s = s2p.tile([P, 8, GD], F32R)
                    # per-chunk loads so stage-2 matmuls start after ~1.3MB
                    xt_r = xt.rearrange("(c p) t -> c p t", p=P)
                    wq_r = wq.rearrange("(c p) n -> c p n", p=P)
                    wk_r = wk.rearrange("(c p) n -> c p n", p=P)
                    wv_r = wv.rearrange("(c p) n -> c p n", p=P)
                    for c in range(8):
                        nc.sync.dma_start(xt_s[:, c], xt_r[c])
                        nc.sync.dma_start(wq_s[:, c], wq_r[c])
                        nc.sync.dma_start(wk_s[:, c], wk_r[c])
                        nc.sync.dma_start(wv_s[:, c], wv_r[c])

                    # c-outer: accumulate all 8 (J,t) q/k tiles as chunks arrive
                    qk_ps = {}
                    for J in range(2):
                        for t in range(4):
                            for i in range(2):
                                qk_ps[(J, t, i)] = ps2p.tile([P, 512], F32, tag="qk", name=f"qkps_{J}_{t}_{i}")
                    for c in range(8):
                        for J in range(2):
                            for t in range(4):
                                for i, w_s in enumerate((wq_s, wk_s)):
                                    nc.tensor.matmul(
                                        qk_ps[(J, t, i)][:],
                                        w_s[:, c, J * P:(J + 1) * P],
                                        xt_s[:, c, t * 512:(t + 1) * 512],
                                        start=(c == 0), stop=(c == 7),
                                    )
                    for J in range(2):
                        for t in range(4):
                            tsl = slice(t * 512, (t + 1) * 512)
                            nc.scalar.activation(
                                qT_s[:, J, tsl], qk_ps[(J, t, 0)][:],
                                AF.Identity, bias=bq_s[:, J:J + 1],
                            )
                            # kT per-head padded blocks: data rows + zero rows
                            psk = qk_ps[(J, t, 1)]
                            nc.scalar.activation(
                                kT_z[0:64, 2 * J, tsl], psk[0:64, :],
                                AF.Identity, bias=bk_s[0:64, J:J + 1],
                            )
                            nc.scalar.activation(
                                kT_z[64:P, 2 * J + 1, tsl], psk[64:P, :],
                                AF.Identity, bias=bk_s[64:P, J:J + 1],
                            )
                            nc.vector.tensor_scalar_mul(
                                kT_z[64:P, 2 * J, tsl], psk[64:P, :], 0.0)
                            nc.vector.tensor_scalar_mul(
                                kT_z[0:64, 2 * J + 1, tsl], psk[0:64, :], 0.0)
                    for t in range(16):
                        ps = ps2p.tile([P, 512], F32, tag="qk")
                        psv = ps[:, 0:GD]
                        for c in range(8):
                            nc.tensor.matmul(
                                psv,
                                xt_s[:, c, t * P:(t + 1) * P],
                                wv_s[:, c, :],
                                start=(c == 0), stop=(c == 7),
                            )
                        nc.vector.tensor_copy(
                            vaug_s.rearrange("p t (h c) -> p t h c", c=65)[:, t, :, 0:64],
                            psv.rearrange("p (h c) -> p h c", c=64),
                        )

                # ---------------- stage 3 + 4 ----------------
                with (
                    tc.tile_pool(name="slab", bufs=20) as slabp,
                    tc.tile_pool(name="misc", bufs=3) as miscp,
                    tc.tile_pool(name="pss", bufs=2, space="PSUM") as pssp,
                    tc.tile_pool(name="pso", bufs=2, space="PSUM") as psop,
                    tc.tile_pool(name="psb", bufs=1, space="PSUM") as psbp,
                    tc.tile_pool(name="ps4", bufs=1, space="PSUM") as ps4p,
                ):
                    for qt in range(4):
                        q0 = qt * 512
                        for h in range(4):
                            hj = h // 2
                            ps_o = psop.tile([65, 512], F32)
                            g_tiles = []
                            for g in range(8):
                                ps_s = pssp.tile([P, 1024], F32)
                                for j2 in range(2):
                                    kt = g * 2 + j2
                                    nc.tensor.matmul(
                                        ps_s[:, j2 * 512:(j2 + 1) * 512],
                                        kT_z[:, h, kt * P:(kt + 1) * P],
                                        qT_s[:, hj, q0:q0 + 512],
                                        start=True, stop=True,
                                    )
                                eg = slabp.tile([P, 1024], F16, tag="slab")
                                nc.scalar.activation(eg[:], ps_s[:], AF.Exp, scale=0.125)
                                g_tiles.append(eg)
                                for j2 in range(2):
                                    kt = g * 2 + j2
                                    nc.tensor.matmul(
                                        ps_o[:],
                                        vaug_s[:, kt, h * 65:(h + 1) * 65],
                                        eg[:, j2 * 512:(j2 + 1) * 512],
                                        start=(kt == 0), stop=(kt == 15),
                                    )
                            # Z -> recip -> K=1 matmul broadcast to all partitions
                            rc = miscp.tile([P, 512], F32R, tag="rc")
                            with nc.allow_low_precision(reason="recip feeds fp32r bcast matmul"):
                                nc.vector.reciprocal(rc[64:65, :], ps_o[64:65, :])
                            bc_ps = psbp.tile([P, 512], F32)
                            nc.tensor.matmul(bc_ps[:], ones_s[64:65, :], rc[64:65, :],
                                             start=True, stop=True)
                            bc = miscp.tile([P, 512], F16, tag="bc")
                            nc.scalar.copy(bc[:], bc_ps[:])
                            # normalized attn-out^T column block
                            if h % 2 == 0:
                                nc.vector.tensor_tensor(
                                    att_s[0:64, h // 2, q0:q0 + 512],
                                    ps_o[0:64, :], bc[0:64, :],
                                    op=mybir.AluOpType.mult,
                                )
                            else:
                                odd_tmp = miscp.tile([64, 512], F32R, tag="odd")
                                nc.vector.tensor_tensor(
                                    odd_tmp[:], ps_o[0:64, :], bc[0:64, :],
                                    op=mybir.AluOpType.mult,
                                )
                                nc.sync.dma_start(
                                    att_s[64:P, h // 2, q0:q0 + 512], odd_tmp[:])
                            # normalize attn in place and write out
                            # (split across DVE and GPSIMD to balance engines)
                            for g in range(8):
                                eg = g_tiles[g]
                                eng = nc.gpsimd if g % 3 == 2 else nc.vector
                                for j2 in range(2):
                                    kt = g * 2 + j2
                                    eng.tensor_tensor(
                                        eg[:, j2 * 512:(j2 + 1) * 512],
                                        eg[:, j2 * 512:(j2 + 1) * 512],
                                        bc[:],
                                        op=mybir.AluOpType.mult,
                                    )
                                    nc.sync.dma_start(
                                        attnT[h, kt * P:(kt + 1) * P, q0:q0 + 512],
                                        eg[:, j2 * 512:(j2 + 1) * 512],
                                    )
                        # ---- stage 4 for this q tile ----
                        for qs in range(4):
                            for n2 in range(2):
                                ps4 = ps4p.tile([P, 512], F32)
                                for hj in range(2):
                                    nc.tensor.matmul(
                                        ps4[:],
                                        att_s[:, hj, q0 + qs * P:q0 + (qs + 1) * P],
                                        wo_s[:, hj, n2 * 512:(n2 + 1) * 512],
                                        start=(hj == 0), stop=(hj == 1),
                                    )
                                st = miscp.tile([P, 512], F32, tag="st")
                                nc.vector.tensor_copy(st[:], ps4[:])
                                nc.sync.dma_start(
                                    outp[q0 + qs * P:q0 + (qs + 1) * P,
                                         n2 * 512:(n2 + 1) * 512],
                                    st[:],
                                )

        if loop_iters is not None and loop_iters > 1:
            with tc.For_i(0, loop_iters, 1):
                body()
        else:
            body()

    nc.compile()
    return nc


_cache: dict = {}


def _get_nc(loop_iters=None):
    key = loop_iters
    if key not in _cache:
        _cache[key] = _build(loop_iters)
    return _cache[key]


def _make_in_maps(x, wq, bq, wk, bk, wv, wo):
    in_maps = []
    for c in range(8):
        b, g = c // 4, c % 4
        sl = slice(g * GD, (g + 1) * GD)
        in_maps.append({
            "xt": np.ascontiguousarray(x[b].T),
            "wq": np.ascontiguousarray(wq[:, sl]),
            "wk": np.ascontiguousarray(wk[:, sl]),
            "wv": np.ascontiguousarray(wv[:, sl]),
            "wo": np.ascontiguousarray(wo[sl, :]),
            "bq": np.ascontiguousarray(bq[sl].reshape(2, P).T),
            "bk": np.ascontiguousarray(bk[sl].reshape(2, P).T),
        })
    return in_maps


def kernel(x, wq, bq, wk, bk, wv, bv, wo, bo):
    x = np.asarray(x, dtype=np.float32)
    nc = _get_nc()
    in_maps = _make_in_maps(x, wq, bq, wk, bk, wv, wo)
    res = run_bass_kernel_spmd(nc, in_maps, list(range(8))).results

    out = np.zeros((B, S, D), dtype=np.float32)
    attn = np.empty((B, H, S, S), dtype=np.float32)
    for c in range(8):
        b, g = c // 4, c % 4
        out[b] += res[c]["outp"]
        attn[b, g * HPC:(g + 1) * HPC] = res[c]["attnT"].transpose(0, 2, 1)
    out += np.asarray(bv, np.float32) @ np.asarray(wo, np.float32) + np.asarray(bo, np.float32)
    return out, attn
